# revision 18
# baseline (speedup 1.0000x reference)
"""Chamfer loss (whole-image) on 8 Trainium2 NeuronCores.

Math (matches the reference):
  p: N=16384 render points (img_render_points.reshape(-1, 2)).
  q: M=20736 grid points (y=10j, x=10i), i<192, j<108, m = i*108 + j.
  out = sum_n min_m ||p_n - q_m|| + sum_m min_n ||p_n - q_m||.

Strategy:
  * Row-min side (min over the grid) is separable because the grid is a
    Cartesian product: min_m d2 = min_i (px-10i)^2 + min_j (py-10j)^2,
    and for |p| < 10 the nearest axis value is 0 or 10 -> closed form.
  * Col-min side is dense: the grid axis (padded to 21504) is sharded
    across the 8 cores (2688 grid points = 21 partition-tiles per core);
    each core scans all N=16384 points. d2 is produced on the
    TensorEngine with a K=11 matmul: d2[m,n] = qx*(-2px) + qy*(-2py)
    + q2 + p2, each fp32 factor expanded into bf16 hi/lo terms (bf16
    products are exact in fp32, PSUM accumulates fp32 -> fp32-accurate
    at the bf16 1-cycle/column matmul rate). Since K=11 <= 32, four
    m-tiles are packed into distinct PE row-groups (tile_position) and
    their matmuls execute concurrently.
  * The min over n runs on the VectorEngine via a custom fused DVE op
    (min(Src0,Src1) with a MIN accumulator seeded from the previous
    call) that ingests two streams per cycle: one bank directly from
    PSUM, one staged to SBUF by the ScalarEngine.
  * Host applies the final sqrt (monotonic, commutes with min) and sums.
"""

import numpy as np
import ml_dtypes

import concourse.bacc as bacc
import concourse.bass as bass
import concourse.mybir as mybir
import concourse.tile as tile
from concourse.tile import add_dep_helper
from concourse import bass_utils
from concourse._compat import get_trn_type

BF16 = ml_dtypes.bfloat16


def _ensure_ntff_hook():
    """This image's `antenv` lacks `axon_hooks`, which run_bass_kernel_spmd
    imports when trace=True. Install an equivalent shim backed by the ctypes
    NTFF driver from trn_agent_boot. Best-effort: failures leave tracing off."""
    try:
        import antenv  # noqa: F401
        try:
            from antenv.axon_hooks import get_axon_ntff_profile_hook  # noqa: F401
            return  # real module exists
        except ImportError:
            pass
        import os
        import sys
        import types

        from trn_agent_boot.trn_boot import _ntff_profile_via_ctypes

        mod = types.ModuleType("antenv.axon_hooks")
        _state = {"hook": None}
        mod.set_axon_ntff_profile_hook = lambda h: _state.__setitem__("hook", h)
        mod.get_axon_ntff_profile_hook = lambda: _state["hook"]
        sys.modules["antenv.axon_hooks"] = mod
        antenv.axon_hooks = mod
        so = "/opt/axon/libaxon_pjrt.so"
        if os.path.exists(so):
            mod.set_axon_ntff_profile_hook(_ntff_profile_via_ctypes(so))
    except Exception:
        pass


_ensure_ntff_hook()

# Problem constants (hardcoded: harness runs kernel.py standalone).
H, W, STRIDE = 1080, 1920, 10
NY = -(-H // STRIDE)        # 108 grid y-values
NX = -(-W // STRIDE)        # 192 grid x-values
M = NX * NY                 # 20736 grid points
N = 128 * 128               # 16384 render points
NCORES = 8
MT = 21                     # m-tiles (128 wide) per core
M_LOC = 128 * MT            # 2688 grid points per core (padded total 21504)
N_LOC = N // NCORES         # 2048 rowmin points per core
K = 11                      # bf16 split-precision contraction rows
NCHUNK = N // 512           # 32 moving-operand chunks of 512
GRPS = (MT + 1) // 2        # 11 groups of (up to) 2 row-alternated m-tiles
FMAX = float(np.finfo(np.float32).max)
GRID_MAX_X = float(STRIDE * (NX - 1))
GRID_MAX_Y = float(STRIDE * (NY - 1))

_minmin_op = None
_built = None


def _get_minmin_op():
    """Register (once) the fused DVE op: out = min(in0, in1),
    accum_out = min(s0, min over free dim of out)."""
    global _minmin_op
    if _minmin_op is not None:
        return _minmin_op
    from concourse import dve_ops as DO
    from concourse.dve_spec import Spec, Src0, Src1, C0, minn, lower
    from concourse.dve_uop import DveOpSpec

    name = "TENSOR_MIN_MIN_REDUCE_CHAMFER"
    for op in DO.OPS:
        if op.name == name:
            _minmin_op = op
            return op

    def _ref(in0, in1, c0, c1, c2):
        b = np.minimum(in0.astype(np.float32), in1.astype(np.float32))
        acc = np.minimum(
            np.asarray(c0, np.float32).reshape(-1, 1),
            b.reshape(b.shape[0], -1).min(axis=-1, keepdims=True),
        )
        return b, acc

    spec = Spec(body=minn(Src0, Src1), accum=minn, accum_init=C0, reference=_ref)
    opcode = DO._CUSTOM_DVE_ROW_BASE + len(DO.OPS)
    assert opcode < 0x20, "custom-DVE opcode rows exhausted"
    shas = {}
    for ver in ("v3", "v4"):
        s = DveOpSpec(name=name, opcode=opcode, uops=lower(spec, ver=ver), rd1_en=True)
        shas[ver] = s.sha(ver)
    op = DO.DveOp(name, spec, subdim=False, uops_sha=shas)
    DO.OPS.append(op)
    DO._SUB_OPCODE_FOR_NAME[name] = opcode
    DO.CUSTOM_DVE_SPECS[name] = spec
    _minmin_op = op
    return op


def _build():
    """Trace + compile the per-core Bass kernel once."""
    global _built
    if _built is not None:
        return _built
    op = _get_minmin_op()
    nc = bacc.Bacc(get_trn_type() or "TRN2", target_bir_lowering=False, debug=False)
    f32 = mybir.dt.float32
    bf16 = mybir.dt.bfloat16
    ACT = mybir.ActivationFunctionType
    ALU = mybir.AluOpType

    # Dense 128-partition layout: rows 32h+k hold band h's K-row k (other
    # rows zero -- DMA bandwidth scales with partition coverage, not bytes).
    # Columns: [lhsT (GRPS*128) | rhs (N)]. Three column-chunk DMAs on the
    # three DMA-capable queues; the first chunk carries lhsT + early rhs so
    # matmuls start as soon as it lands.
    QPC = GRPS * 128 + N
    qp_d = nc.dram_tensor("qp", (128, QPC), bf16, kind="ExternalInput")
    prow_d = nc.dram_tensor("prow", (128, 2, 16), f32, kind="ExternalInput")
    col_d = nc.dram_tensor("colmin_sq", (128, MT), f32, kind="ExternalOutput")
    row_d = nc.dram_tensor("rowmin_sq", (128, 16), f32, kind="ExternalOutput")

    with tile.TileContext(nc) as tc:
        with (
            tc.tile_pool(name="const", bufs=1) as cpool,
            tc.tile_pool(name="stage", bufs=4) as spool,
            tc.tile_pool(name="scr", bufs=2) as gpool,
            tc.tile_pool(name="accs", bufs=12) as apool,
            tc.tile_pool(name="rmin", bufs=1) as rpool,
            tc.tile_pool(name="ps", bufs=4, space=bass.MemorySpace.PSUM) as pspool,
        ):
            prow = cpool.tile([128, 2, 16], f32)
            dprow = nc.sync.dma_start(prow[:], prow_d[:])
            qp = cpool.tile([128, QPC], bf16)
            c1 = GRPS * 128 + 4096
            c2 = c1 + (QPC - c1) // 2
            d1 = nc.sync.dma_start(qp[:, :c1], qp_d[:, :c1])
            add_dep_helper(d1.ins, dprow.ins, False, "prow first on sync queue")
            nc.scalar.dma_start(qp[:, c1:c2], qp_d[:, c1:c2])
            nc.gpsimd.dma_start(qp[:, c2:], qp_d[:, c2:])
            lhsT = qp[:, : GRPS * 128]

            def rhs_ap(rg, n0, n1):
                return qp[rg : rg + K, GRPS * 128 + n0 : GRPS * 128 + n1]
            colout = cpool.tile([128, MT], f32)
            rowout = cpool.tile([128, 16], f32)

            # ---- row-min side: nearest axis value for |v|<10 is 0 or 10 ----
            q0 = rpool.tile([128, 2, 16], f32)
            nc.vector.scalar_tensor_tensor(
                out=q0[:], in0=prow[:], scalar=0.0, in1=prow[:],
                op0=ALU.add, op1=ALU.mult,
            )
            tshift = rpool.tile([128, 2, 16], f32)
            nc.vector.tensor_scalar_add(tshift[:], prow[:], -float(STRIDE))
            q1 = rpool.tile([128, 2, 16], f32)
            nc.vector.scalar_tensor_tensor(
                out=q1[:], in0=tshift[:], scalar=0.0, in1=tshift[:],
                op0=ALU.add, op1=ALU.mult,
            )
            qm = rpool.tile([128, 2, 16], f32)
            nc.vector.scalar_tensor_tensor(
                out=qm[:], in0=q0[:], scalar=0.0, in1=q1[:],
                op0=ALU.add, op1=ALU.min,
            )
            nc.vector.scalar_tensor_tensor(
                out=rowout[:], in0=qm[:, 0, :], scalar=0.0, in1=qm[:, 1, :],
                op0=ALU.add, op1=ALU.add,
            )
            nc.sync.dma_start(row_d[:], rowout[:])

            # ---- col-min side ----
            # Per group: two m-tiles on alternating PE row-groups (0 / 32) so
            # LDWEIGHTS+MATMUL chains of one overlap the other's. Each PSUM
            # tile holds 4 chunks (2048 n) of ONE m-tile; even tiles are
            # staged to SBUF by ScalarE, odd tiles are read directly by the
            # fused VectorE min op (FD=2048 per call).
            prev_mm = None
            for grp in range(GRPS):
                tlist = [t for t in (2 * grp, 2 * grp + 1) if t < MT]
                accs = {t: None for t in tlist}
                S_tiles = {}
                nt = len(tlist)
                for u in range(16 * nt):
                    t = tlist[u % nt]
                    s = t % 2
                    tiw = u // nt           # 16 two-chunk tiles per m-tile
                    # 2-bank PSUM tile (chunks 2*tiw, 2*tiw+1 of m-tile t);
                    # bufs=4 so PE fill, ScalarE copy and VectorE min all
                    # overlap. Row-group alternates every matmul (weights
                    # replicated at rg 32*s and 32*s+64).
                    P = pspool.tile([128, 2, 512], f32, tag="P", name="P")
                    for c in range(2):
                        chunk = 2 * tiw + c
                        rg = 32 * s + 64 * (chunk % 2)
                        n0 = chunk * 512
                        mm = nc.tensor.matmul(
                            P[:, c, :],
                            lhsT[rg : rg + K, grp * 128 : (grp + 1) * 128],
                            rhs_ap(rg, n0, n0 + 512),
                            tile_position=(rg, 0),
                        )
                        if prev_mm is not None:
                            add_dep_helper(mm.ins, prev_mm.ins, False,
                                           "rg-alternating PE order")
                        prev_mm = mm
                    if tiw % 2 == 0:
                        S = spool.tile([128, 2, 512], f32, tag="S", name="S")
                        nc.scalar.activation(S[:], P[:], ACT.Copy)
                        S_tiles[t] = S
                    else:
                        garbage = gpool.tile([128, 2, 512], f32, tag="garb", name="garb")
                        if tiw == 15:
                            acc_out = colout[:, t : t + 1]
                        else:
                            acc_out = apool.tile([128, 1], f32, tag="acc", name="acc")[:]
                        last_custom = nc.vector._custom_dve(
                            op,
                            out=garbage[:],
                            in0=P[:],
                            in1=S_tiles[t][:],
                            s0=(FMAX if accs[t] is None else accs[t]),
                            accum_out=acc_out,
                        )
                        accs[t] = acc_out
            nc.sync.dma_start(col_d[:], colout[:])

    nc.compile()
    _built = nc
    return nc


def _split_bf16(v, n_terms):
    """Split float64 array into n_terms bf16 arrays with sum ~= v."""
    parts = []
    r = np.asarray(v, np.float64).copy()
    for _ in range(n_terms):
        p = r.astype(BF16)
        parts.append(p)
        r -= p.astype(np.float64)
    return parts


# Results of the most recent device run (exec_time_ns etc.), for test harnesses.
LAST_RUN = None


def kernel(img_render_points, img_ref):
    assert img_ref.shape == (H, W), f"unexpected img_ref shape {img_ref.shape}"
    p = np.asarray(img_render_points, np.float32).reshape(-1, 2).astype(np.float64)
    assert p.shape[0] == N
    # Closed-form row-min assumes the nearest grid axis value is 0 or STRIDE.
    assert np.abs(p).max() < STRIDE, "render points exceed closed-form row-min range"

    pa = p[:, 0]  # pairs with grid y = 10j
    pb = p[:, 1]  # pairs with grid x = 10i

    # q-side (lhsT): padded grid, sharded across cores, 4-way row-packed.
    M_PAD = M_LOC * NCORES
    m = np.arange(M_PAD)
    i = np.where(m < M, m // NY, 0)
    j = np.where(m < M, m % NY, 0)
    qb = (STRIDE * i).astype(np.float64)  # x
    qa = (STRIDE * j).astype(np.float64)  # y
    q2 = qa * qa + qb * qb
    qb_h, qb_l = _split_bf16(qb, 2)
    qa_h, qa_l = _split_bf16(qa, 2)
    q2_h, q2_m, q2_l = _split_bf16(q2, 3)
    ones_m = np.ones(M_PAD, BF16)
    lhsT_rows = np.stack(
        [qb_h, qb_h, qb_l, qa_h, qa_h, qa_l, q2_h, q2_m, q2_l, ones_m, ones_m]
    )  # (K, M_PAD) bf16

    # p-side (rhs): shared by all cores, replicated into 4 row-groups.
    b_h, b_l = _split_bf16(-2.0 * pb, 2)
    a_h, a_l = _split_bf16(-2.0 * pa, 2)
    p2_h, p2_l = _split_bf16(pa * pa + pb * pb, 2)
    ones_n = np.ones(N, BF16)
    rhs_rows = np.stack(
        [b_h, b_l, b_h, a_h, a_l, a_h, ones_n, ones_n, ones_n, p2_h, p2_l]
    )  # (K, N) bf16


    in_maps = []
    for c in range(NCORES):
        GCOLS = GRPS * 128
        qp_b = np.zeros((128, GCOLS + N), BF16)
        for h in range(4):
            qp_b[32 * h : 32 * h + K, GCOLS:] = rhs_rows
        base = c * M_LOC
        for t in range(MT):
            grp, s = t // 2, t % 2
            cols = slice(base + t * 128, base + (t + 1) * 128)
            for h in (s, s + 2):
                qp_b[32 * h : 32 * h + K, grp * 128 : (grp + 1) * 128] = lhsT_rows[:, cols]
        pa_c = pa[c * N_LOC : (c + 1) * N_LOC].astype(np.float32).reshape(128, 16)
        pb_c = pb[c * N_LOC : (c + 1) * N_LOC].astype(np.float32).reshape(128, 16)
        in_maps.append(
            {
                "qp": qp_b,
                "prow": np.ascontiguousarray(np.stack([pa_c, pb_c], axis=1)),
            }
        )

    nc = _build()
    global LAST_RUN
    LAST_RUN = bass_utils.run_bass_kernel_spmd(nc, in_maps, core_ids=list(range(NCORES)))

    colmins = np.concatenate(
        [r["colmin_sq"].T.reshape(-1) for r in LAST_RUN.results]
    )[:M]
    rowmins = np.concatenate([r["rowmin_sq"].reshape(-1) for r in LAST_RUN.results])
    total = (
        np.sqrt(np.maximum(colmins, 0.0).astype(np.float64)).sum()
        + np.sqrt(np.maximum(rowmins, 0.0).astype(np.float64)).sum()
    )
    return np.array(total, dtype=np.float32)


# revision 19
# speedup vs baseline: 1.0011x; 1.0011x over previous
"""Chamfer loss (whole-image) on 8 Trainium2 NeuronCores.

Math (matches the reference):
  p: N=16384 render points (img_render_points.reshape(-1, 2)).
  q: M=20736 grid points (y=10j, x=10i), i<192, j<108, m = i*108 + j.
  out = sum_n min_m ||p_n - q_m|| + sum_m min_n ||p_n - q_m||.

Strategy:
  * Row-min side (min over the grid) is separable because the grid is a
    Cartesian product: min_m d2 = min_i (px-10i)^2 + min_j (py-10j)^2,
    and for |p| < 10 the nearest axis value is 0 or 10 -> closed form.
  * Col-min side is dense: the grid axis (padded to 21504) is sharded
    across the 8 cores (2688 grid points = 21 partition-tiles per core);
    each core scans all N=16384 points. d2 is produced on the
    TensorEngine with a K=11 matmul: d2[m,n] = qx*(-2px) + qy*(-2py)
    + q2 + p2, each fp32 factor expanded into bf16 hi/lo terms (bf16
    products are exact in fp32, PSUM accumulates fp32 -> fp32-accurate
    at the bf16 1-cycle/column matmul rate). Since K=11 <= 32, four
    m-tiles are packed into distinct PE row-groups (tile_position) and
    their matmuls execute concurrently.
  * The min over n runs on the VectorEngine via a custom fused DVE op
    (min(Src0,Src1) with a MIN accumulator seeded from the previous
    call) that ingests two streams per cycle: one bank directly from
    PSUM, one staged to SBUF by the ScalarEngine.
  * Host applies the final sqrt (monotonic, commutes with min) and sums.
"""

import numpy as np
import ml_dtypes

import concourse.bacc as bacc
import concourse.bass as bass
import concourse.mybir as mybir
import concourse.tile as tile
from concourse.tile import add_dep_helper
from concourse import bass_utils
from concourse._compat import get_trn_type

BF16 = ml_dtypes.bfloat16


def _ensure_ntff_hook():
    """This image's `antenv` lacks `axon_hooks`, which run_bass_kernel_spmd
    imports when trace=True. Install an equivalent shim backed by the ctypes
    NTFF driver from trn_agent_boot. Best-effort: failures leave tracing off."""
    try:
        import antenv  # noqa: F401
        try:
            from antenv.axon_hooks import get_axon_ntff_profile_hook  # noqa: F401
            return  # real module exists
        except ImportError:
            pass
        import os
        import sys
        import types

        from trn_agent_boot.trn_boot import _ntff_profile_via_ctypes

        mod = types.ModuleType("antenv.axon_hooks")
        _state = {"hook": None}
        mod.set_axon_ntff_profile_hook = lambda h: _state.__setitem__("hook", h)
        mod.get_axon_ntff_profile_hook = lambda: _state["hook"]
        sys.modules["antenv.axon_hooks"] = mod
        antenv.axon_hooks = mod
        so = "/opt/axon/libaxon_pjrt.so"
        if os.path.exists(so):
            mod.set_axon_ntff_profile_hook(_ntff_profile_via_ctypes(so))
    except Exception:
        pass


_ensure_ntff_hook()

# Problem constants (hardcoded: harness runs kernel.py standalone).
H, W, STRIDE = 1080, 1920, 10
NY = -(-H // STRIDE)        # 108 grid y-values
NX = -(-W // STRIDE)        # 192 grid x-values
M = NX * NY                 # 20736 grid points
N = 128 * 128               # 16384 render points
NCORES = 8
MT = 21                     # m-tiles (128 wide) per core
M_LOC = 128 * MT            # 2688 grid points per core (padded total 21504)
N_LOC = N // NCORES         # 2048 rowmin points per core
K = 11                      # bf16 split-precision contraction rows
NCHUNK = N // 512           # 32 moving-operand chunks of 512
GRPS = (MT + 1) // 2        # 11 groups of (up to) 2 row-alternated m-tiles
FMAX = float(np.finfo(np.float32).max)
GRID_MAX_X = float(STRIDE * (NX - 1))
GRID_MAX_Y = float(STRIDE * (NY - 1))

_minmin_op = None
_built = None


def _get_minmin_op():
    """Register (once) the fused DVE op: out = min(in0, in1),
    accum_out = min(s0, min over free dim of out)."""
    global _minmin_op
    if _minmin_op is not None:
        return _minmin_op
    from concourse import dve_ops as DO
    from concourse.dve_spec import Spec, Src0, Src1, C0, minn, lower
    from concourse.dve_uop import DveOpSpec

    name = "TENSOR_MIN_MIN_REDUCE_CHAMFER"
    for op in DO.OPS:
        if op.name == name:
            _minmin_op = op
            return op

    def _ref(in0, in1, c0, c1, c2):
        b = np.minimum(in0.astype(np.float32), in1.astype(np.float32))
        acc = np.minimum(
            np.asarray(c0, np.float32).reshape(-1, 1),
            b.reshape(b.shape[0], -1).min(axis=-1, keepdims=True),
        )
        return b, acc

    spec = Spec(body=minn(Src0, Src1), accum=minn, accum_init=C0, reference=_ref)
    opcode = DO._CUSTOM_DVE_ROW_BASE + len(DO.OPS)
    assert opcode < 0x20, "custom-DVE opcode rows exhausted"
    shas = {}
    for ver in ("v3", "v4"):
        s = DveOpSpec(name=name, opcode=opcode, uops=lower(spec, ver=ver), rd1_en=True)
        shas[ver] = s.sha(ver)
    op = DO.DveOp(name, spec, subdim=False, uops_sha=shas)
    DO.OPS.append(op)
    DO._SUB_OPCODE_FOR_NAME[name] = opcode
    DO.CUSTOM_DVE_SPECS[name] = spec
    _minmin_op = op
    return op


def _build():
    """Trace + compile the per-core Bass kernel once."""
    global _built
    if _built is not None:
        return _built
    op = _get_minmin_op()
    nc = bacc.Bacc(get_trn_type() or "TRN2", target_bir_lowering=False, debug=False)
    f32 = mybir.dt.float32
    bf16 = mybir.dt.bfloat16
    ACT = mybir.ActivationFunctionType
    ALU = mybir.AluOpType

    # Dense 128-partition layout: rows 32h+k hold band h's K-row k (other
    # rows zero -- DMA bandwidth scales with partition coverage, not bytes).
    # Columns: [lhsT (GRPS*128) | rhs (N)]. Three column-chunk DMAs on the
    # three DMA-capable queues; the first chunk carries lhsT + early rhs so
    # matmuls start as soon as it lands.
    QPC = GRPS * 128 + N
    C1 = GRPS * 128 + 4096          # chunk A: lhsT + first 8 rhs chunk-pairs
    C2 = C1 + (QPC - C1) // 2
    qpa_d = nc.dram_tensor("qpa", (128, C1), bf16, kind="ExternalInput")
    qpb_d = nc.dram_tensor("qpb", (128, C2 - C1), bf16, kind="ExternalInput")
    qpc_d = nc.dram_tensor("qpc", (128, QPC - C2), bf16, kind="ExternalInput")
    prow_d = nc.dram_tensor("prow", (128, 2, 16), f32, kind="ExternalInput")
    col_d = nc.dram_tensor("colmin_sq", (128, MT), f32, kind="ExternalOutput")
    row_d = nc.dram_tensor("rowmin_sq", (128, 16), f32, kind="ExternalOutput")

    with tile.TileContext(nc) as tc:
        with (
            tc.tile_pool(name="const", bufs=1) as cpool,
            tc.tile_pool(name="stage", bufs=4) as spool,
            tc.tile_pool(name="scr", bufs=2) as gpool,
            tc.tile_pool(name="accs", bufs=12) as apool,
            tc.tile_pool(name="rmin", bufs=1) as rpool,
            tc.tile_pool(name="ps", bufs=4, space=bass.MemorySpace.PSUM) as pspool,
        ):
            prow = cpool.tile([128, 2, 16], f32)
            dprow = nc.sync.dma_start(prow[:], prow_d[:])
            qp = cpool.tile([128, QPC], bf16)
            d1 = nc.sync.dma_start(qp[:, :C1], qpa_d[:])
            add_dep_helper(d1.ins, dprow.ins, False, "prow first on sync queue")
            nc.scalar.dma_start(qp[:, C1:C2], qpb_d[:])
            nc.gpsimd.dma_start(qp[:, C2:], qpc_d[:])
            lhsT = qp[:, : GRPS * 128]

            def rhs_ap(rg, n0, n1):
                return qp[rg : rg + K, GRPS * 128 + n0 : GRPS * 128 + n1]
            colout = cpool.tile([128, MT], f32)
            rowout = cpool.tile([128, 16], f32)

            # ---- row-min side: nearest axis value for |v|<10 is 0 or 10 ----
            q0 = rpool.tile([128, 2, 16], f32)
            nc.vector.scalar_tensor_tensor(
                out=q0[:], in0=prow[:], scalar=0.0, in1=prow[:],
                op0=ALU.add, op1=ALU.mult,
            )
            tshift = rpool.tile([128, 2, 16], f32)
            nc.vector.tensor_scalar_add(tshift[:], prow[:], -float(STRIDE))
            q1 = rpool.tile([128, 2, 16], f32)
            nc.vector.scalar_tensor_tensor(
                out=q1[:], in0=tshift[:], scalar=0.0, in1=tshift[:],
                op0=ALU.add, op1=ALU.mult,
            )
            qm = rpool.tile([128, 2, 16], f32)
            nc.vector.scalar_tensor_tensor(
                out=qm[:], in0=q0[:], scalar=0.0, in1=q1[:],
                op0=ALU.add, op1=ALU.min,
            )
            nc.vector.scalar_tensor_tensor(
                out=rowout[:], in0=qm[:, 0, :], scalar=0.0, in1=qm[:, 1, :],
                op0=ALU.add, op1=ALU.add,
            )
            nc.sync.dma_start(row_d[:], rowout[:])

            # ---- col-min side ----
            # Per group: two m-tiles on alternating PE row-groups (0 / 32) so
            # LDWEIGHTS+MATMUL chains of one overlap the other's. Each PSUM
            # tile holds 4 chunks (2048 n) of ONE m-tile; even tiles are
            # staged to SBUF by ScalarE, odd tiles are read directly by the
            # fused VectorE min op (FD=2048 per call).
            prev_mm = None
            for grp in range(GRPS):
                tlist = [t for t in (2 * grp, 2 * grp + 1) if t < MT]
                accs = {t: None for t in tlist}
                S_tiles = {}
                nt = len(tlist)
                for u in range(16 * nt):
                    t = tlist[u % nt]
                    s = t % 2
                    tiw = u // nt           # 16 two-chunk tiles per m-tile
                    # 2-bank PSUM tile (chunks 2*tiw, 2*tiw+1 of m-tile t);
                    # bufs=4 so PE fill, ScalarE copy and VectorE min all
                    # overlap. Row-group alternates every matmul (weights
                    # replicated at rg 32*s and 32*s+64).
                    P = pspool.tile([128, 2, 512], f32, tag="P", name="P")
                    for c in range(2):
                        chunk = 2 * tiw + c
                        rg = 32 * s + 64 * (chunk % 2)
                        n0 = chunk * 512
                        mm = nc.tensor.matmul(
                            P[:, c, :],
                            lhsT[rg : rg + K, grp * 128 : (grp + 1) * 128],
                            rhs_ap(rg, n0, n0 + 512),
                            tile_position=(rg, 0),
                        )
                        if prev_mm is not None:
                            add_dep_helper(mm.ins, prev_mm.ins, False,
                                           "rg-alternating PE order")
                        prev_mm = mm
                    if tiw % 2 == 0:
                        S = spool.tile([128, 2, 512], f32, tag="S", name="S")
                        nc.scalar.activation(S[:], P[:], ACT.Copy)
                        S_tiles[t] = S
                    else:
                        garbage = gpool.tile([128, 2, 512], f32, tag="garb", name="garb")
                        if tiw == 15:
                            acc_out = colout[:, t : t + 1]
                        else:
                            acc_out = apool.tile([128, 1], f32, tag="acc", name="acc")[:]
                        last_custom = nc.vector._custom_dve(
                            op,
                            out=garbage[:],
                            in0=P[:],
                            in1=S_tiles[t][:],
                            s0=(FMAX if accs[t] is None else accs[t]),
                            accum_out=acc_out,
                        )
                        accs[t] = acc_out
            nc.sync.dma_start(col_d[:], colout[:])

    nc.compile()
    _built = nc
    return nc


def _split_bf16(v, n_terms):
    """Split float64 array into n_terms bf16 arrays with sum ~= v."""
    parts = []
    r = np.asarray(v, np.float64).copy()
    for _ in range(n_terms):
        p = r.astype(BF16)
        parts.append(p)
        r -= p.astype(np.float64)
    return parts


# Results of the most recent device run (exec_time_ns etc.), for test harnesses.
LAST_RUN = None


def kernel(img_render_points, img_ref):
    assert img_ref.shape == (H, W), f"unexpected img_ref shape {img_ref.shape}"
    p = np.asarray(img_render_points, np.float32).reshape(-1, 2).astype(np.float64)
    assert p.shape[0] == N
    # Closed-form row-min assumes the nearest grid axis value is 0 or STRIDE.
    assert np.abs(p).max() < STRIDE, "render points exceed closed-form row-min range"

    pa = p[:, 0]  # pairs with grid y = 10j
    pb = p[:, 1]  # pairs with grid x = 10i

    # q-side (lhsT): padded grid, sharded across cores, 4-way row-packed.
    M_PAD = M_LOC * NCORES
    m = np.arange(M_PAD)
    i = np.where(m < M, m // NY, 0)
    j = np.where(m < M, m % NY, 0)
    qb = (STRIDE * i).astype(np.float64)  # x
    qa = (STRIDE * j).astype(np.float64)  # y
    q2 = qa * qa + qb * qb
    qb_h, qb_l = _split_bf16(qb, 2)
    qa_h, qa_l = _split_bf16(qa, 2)
    q2_h, q2_m, q2_l = _split_bf16(q2, 3)
    ones_m = np.ones(M_PAD, BF16)
    lhsT_rows = np.stack(
        [qb_h, qb_h, qb_l, qa_h, qa_h, qa_l, q2_h, q2_m, q2_l, ones_m, ones_m]
    )  # (K, M_PAD) bf16

    # p-side (rhs): shared by all cores, replicated into 4 row-groups.
    b_h, b_l = _split_bf16(-2.0 * pb, 2)
    a_h, a_l = _split_bf16(-2.0 * pa, 2)
    p2_h, p2_l = _split_bf16(pa * pa + pb * pb, 2)
    ones_n = np.ones(N, BF16)
    rhs_rows = np.stack(
        [b_h, b_l, b_h, a_h, a_l, a_h, ones_n, ones_n, ones_n, p2_h, p2_l]
    )  # (K, N) bf16


    in_maps = []
    for c in range(NCORES):
        GCOLS = GRPS * 128
        qp_b = np.zeros((128, GCOLS + N), BF16)
        for h in range(4):
            qp_b[32 * h : 32 * h + K, GCOLS:] = rhs_rows
        base = c * M_LOC
        for t in range(MT):
            grp, s = t // 2, t % 2
            cols = slice(base + t * 128, base + (t + 1) * 128)
            for h in (s, s + 2):
                qp_b[32 * h : 32 * h + K, grp * 128 : (grp + 1) * 128] = lhsT_rows[:, cols]
        pa_c = pa[c * N_LOC : (c + 1) * N_LOC].astype(np.float32).reshape(128, 16)
        pb_c = pb[c * N_LOC : (c + 1) * N_LOC].astype(np.float32).reshape(128, 16)
        GC1 = GCOLS + 4096
        GC2 = GC1 + (GCOLS + N - GC1) // 2
        in_maps.append(
            {
                "qpa": np.ascontiguousarray(qp_b[:, :GC1]),
                "qpb": np.ascontiguousarray(qp_b[:, GC1:GC2]),
                "qpc": np.ascontiguousarray(qp_b[:, GC2:]),
                "prow": np.ascontiguousarray(np.stack([pa_c, pb_c], axis=1)),
            }
        )

    nc = _build()
    global LAST_RUN
    LAST_RUN = bass_utils.run_bass_kernel_spmd(nc, in_maps, core_ids=list(range(NCORES)))

    colmins = np.concatenate(
        [r["colmin_sq"].T.reshape(-1) for r in LAST_RUN.results]
    )[:M]
    rowmins = np.concatenate([r["rowmin_sq"].reshape(-1) for r in LAST_RUN.results])
    total = (
        np.sqrt(np.maximum(colmins, 0.0).astype(np.float64)).sum()
        + np.sqrt(np.maximum(rowmins, 0.0).astype(np.float64)).sum()
    )
    return np.array(total, dtype=np.float32)
